# revision 7
# baseline (speedup 1.0000x reference)
"""EWMA predictor (sliding-window variance, exponentially weighted sum) on 8 trn2 cores.

Math: for j in [0, L): window_j = x[j : j+128], weight ff^(L-1-j),
result = norm * sum_j ff^(L-1-j) * var(window_j, ddof=1),
norm = (1-ff)/(1-ff^L), ff = sigmoid(raw_forgetting_factor).

Sharding: windows split over 8 cores x 128 partitions; partition p of core c
owns `run` consecutive windows and loads the run+127 input elements covering
them (halo overlap) plus a per-window weight row
WT[p, t] = ff^(i0(c,p) + run-1-t) / 127 in the trailing columns.

Device program (per core):
  input:  SWDGE dma_gather; row indices built on-device (Pool iotas + DVE
          int32 and/add/cast, replicated in every 16-partition group since
          each Q7 core reads the wrapped idxs from its own group), the
          descriptors prepared and triggered early so the completion
          latency overlaps the whole compute chain.
  chain:  all on DVE: X2 = x^2; one fused [128,2,W] reduce seeds both
          sliding-sum recurrences; two tensor_tensor_scan ops produce the
          128-window sums s1/s2; d = s2 - s1^2/128 (= 127*var_unbiased);
          contrib[p] = sum_t WT[p,t]*d[p,t] (elementwise mult + reduce).
  output: SWDGE dma_scatter_add writes contrib[p] into acc[p,0] (stride
          256B rows; output DRAM buffers are zero-initialized by both the
          native runner and the bass2jax/PJRT path).
Host sums the 8x128 partials and applies norm in float64.
"""

import numpy as np

import concourse.bass as bass
import concourse.mybir as mybir
from concourse import library_config
from concourse.library_overlay import lower_extended_insts
from concourse.bass_utils import run_bass_kernel_spmd

L = 524288          # look-back windows
W = 128             # variance window length
N = L + W           # input length
NCORES = 8
WIN_PER_CORE = L // NCORES      # 65536
RUN = WIN_PER_CORE // 128       # 512 windows per partition (full computation)

_NC_CACHE = {}


def plan_run(ff64: float) -> int:
    """Windows-per-partition for the adaptive program.

    Weights ff^i are EXACTLY zero in fp32 (past subnormals) once
    i > 104/|ln ff|, so windows beyond that cannot affect any output bit.
    Keep a >=1024-window margin, round the 1024*run window count up to a
    power-of-two run, clamp to [8, 512]; run=512 is the exact full
    computation (all L windows).
    """
    lnff = np.log(np.float64(ff64))
    if not (lnff < -1e-9):
        return RUN
    k_needed = 104.0 / (-lnff)
    run_min = int(np.ceil((k_needed + 1024.0) / 1024.0))
    run = 8
    while run < run_min:
        run *= 2
    return min(run, RUN)


def geom(run: int) -> tuple[int, int]:
    cols = run + W - 1
    width = (cols + run + 63) // 64 * 64
    return cols, width


def build_nc(run: int = 8) -> bass.Bass:
    cols, width = geom(run)
    nc = bass.Bass(trn_type="TRN2")
    f32 = mybir.dt.float32
    i32 = mybir.dt.int32
    i16 = mybir.dt.int16
    A = mybir.AluOpType
    xt = nc.declare_dram_parameter("xt", [128, width], f32, isOutput=False)
    acc = nc.declare_dram_parameter("acc", [128, 64], f32, isOutput=True)

    with (
        nc.sbuf_tensor([128, 2 * width], f32) as XX,
        nc.sbuf_tensor([128, 2 * run], f32) as S,
        nc.sbuf_tensor([128, run], f32) as D,
        nc.sbuf_tensor([128, run], f32) as E,
        nc.sbuf_tensor([128, 1], f32) as CONTRIB,
        nc.sbuf_tensor([128, 8], i32) as C32,
        nc.sbuf_tensor([128, 1], i32) as P32,
        nc.sbuf_tensor([128, 8], i32) as V32,
        nc.sbuf_tensor([128, 8], i16) as IDX,
        nc.sbuf_tensor([128, 150], f32) as DLY,
        nc.sbuf_tensor([1, 128], f32) as PDLY,
        nc.semaphore() as gsem,    # gather-in completion (DMA sem)
        nc.semaphore() as scsem,   # scatter-out completion (DMA sem)
        nc.semaphore() as psem,    # pool setup progress
        nc.semaphore() as esem,    # gather trigger enqueued
        nc.semaphore() as vsem,    # DVE progress (idx build + chain)
        nc.Block() as block,
    ):
        X = XX[:, 0:width]
        X2 = XX[:, width : width + cols]
        WT = XX[:, cols : cols + run]
        S1 = S[:, 0:run]
        S2 = S[:, run : 2 * run]

        @block.gpsimd
        def _(gpsimd):
            # Gather row index ingredients: IDX[p, c] must be (p%16) + 16c in
            # every 16-partition group (each Q7 core reads the wrapped idxs
            # from its own group; the interpreter reads group 0, hardware
            # group 1). Pool emits the affine parts, DVE the bitwise ones
            # (walrus: int bitwise ops are DVE-only, int16 alu is DVE-only).
            gpsimd.iota(
                C32[:], pattern=[[16, 8]], base=0, channel_multiplier=0
            ).then_inc(psem, 1)
            gpsimd.iota(
                P32[:], pattern=[[0, 1]], base=0, channel_multiplier=1
            ).then_inc(psem, 1)
            gpsimd.memset(PDLY[:, 0:64], 0)
            gpsimd.memset(PDLY[:, 64:128], 0)
            gpsimd.wait_ge(psem, 2)
            gpsimd.wait_ge(vsem, 1)        # P32 &= 15 done on DVE
            gpsimd.tensor_tensor(
                V32[:], C32[:], P32[:].broadcast_to([128, 8]), op=A.add
            ).then_inc(psem, 1)
            gpsimd.wait_ge(psem, 3)
            gpsimd.tensor_copy(IDX[:], V32[:]).then_inc(psem, 1)
            gpsimd.load_library(library_config.mlp)
            gpsimd.wait_ge(psem, 4)
            gpsimd.dma_gather(
                X.unsqueeze(1), xt[:], IDX[:], num_idxs=128, num_idxs_reg=128,
                elem_size=width, prepare_only=True, sem=gsem,
            ).then_inc(psem, 1)
            gpsimd.wait_ge(psem, 5)
            gpsimd.trigger_dma(1)          # fire gather-in
            gpsimd.sem_inc(esem, 1)        # consumers may now wait on gsem
            gpsimd.dma_scatter_add(
                acc[0:128, 0:1], CONTRIB[:], IDX[:], num_idxs=128,
                num_idxs_reg=128, elem_size=1, elem_step=64,
                prepare_only=True, sem=scsem,
            ).then_inc(psem, 1)
            gpsimd.wait_ge(psem, 6)
            gpsimd.wait_ge(vsem, 9)        # contrib ready
            gpsimd.trigger_dma(1)          # fire scatter-out
            gpsimd.wait_ge(scsem, 16)      # real-HW: scatter landed before exit

        @block.vector
        def _(vector):
            # idx build step 1 (bitwise is DVE-only): P32 &= 15
            vector.wait_ge(psem, 2)
            vector.tensor_scalar(
                P32[:], P32[:], 15, None, op0=A.bitwise_and
            ).then_inc(vsem, 1)
            # self-delay sized to arrive at the waits just after the Pool
            # trigger enqueues the gather (eager sem pass); if the trigger is
            # later than this, the blocked wait wakes normally (+100ns).
            vector.memset(DLY[:], 0.0)

            # compute chain
            vector.wait_ge(esem, 1)
            vector.wait_ge(gsem, 16)
            vector.scalar_tensor_tensor(
                X2[:], X[:, 0:cols], 1.0, X[:, 0:cols], op0=A.mult, op1=A.mult
            ).then_inc(vsem, 1)
            vector.wait_ge(vsem, 2)
            # fused initial sums: S[:, 0] = sum x[0:W], S[:, run] = sum x2[0:W]
            vector.reduce_sum(
                S[:].rearrange("p (g r) -> p g r", g=2)[:, :, 0:1],
                XX[:].rearrange("p (g c) -> p g c", g=2)[:, :, 0:W],
                axis=mybir.AxisListType.X,
            ).then_inc(vsem, 1)
            vector.wait_ge(vsem, 3)
            # sliding-sum scans: s[t] = (x[t+W-1] + s[t-1]) - x[t-1]
            vector.tensor_tensor_scan(
                S1[:, 1:run], X[:, W:cols], X[:, 0 : run - 1],
                initial=S1[:, 0:1], op0=A.add, op1=A.subtract,
            ).then_inc(vsem, 1)
            vector.wait_ge(vsem, 4)
            vector.tensor_tensor_scan(
                S2[:, 1:run], X2[:, W:cols], XX[:, width : width + run - 1],
                initial=S2[:, 0:1], op0=A.add, op1=A.subtract,
            ).then_inc(vsem, 1)
            vector.wait_ge(vsem, 5)
            # d = s2 - s1^2/128  (D = (s1 * -1/128) * s1; D = D + s2)
            vector.scalar_tensor_tensor(
                D[:], S1[:], -1.0 / 128.0, S1[:], op0=A.mult, op1=A.mult
            ).then_inc(vsem, 1)
            vector.wait_ge(vsem, 6)
            vector.scalar_tensor_tensor(
                D[:], D[:], 1.0, S2[:], op0=A.mult, op1=A.add
            ).then_inc(vsem, 1)
            vector.wait_ge(vsem, 7)
            # contrib[p] = sum_t WT[p,t] * d[p,t]
            vector.scalar_tensor_tensor(
                E[:], D[:], 1.0, WT[:], op0=A.mult, op1=A.mult
            ).then_inc(vsem, 1)
            vector.wait_ge(vsem, 8)
            vector.reduce_sum(
                CONTRIB[:], E[:], axis=mybir.AxisListType.X
            ).then_inc(vsem, 1)

    lower_extended_insts(nc)  # encode ISA bytes for the NEFF compiler
    return nc


def _get_nc(run: int) -> bass.Bass:
    if run not in _NC_CACHE:
        _NC_CACHE[run] = build_nc(run)
    return _NC_CACHE[run]


def make_in_maps(
    x: np.ndarray, ff32: np.float32, run: int
) -> list[dict[str, np.ndarray]]:
    """Per-core input tiles covering the last 1024*run windows (all L windows
    when run=512); slot (c, p) owns windows starting at
    L - 1024*run + (c*128 + p)*run."""
    cols, width = geom(run)
    start0 = L - 1024 * run
    ff64 = np.float64(ff32)
    p = np.arange(128)
    t = np.arange(run)
    in_maps = []
    for c in range(NCORES):
        base = start0 + c * 128 * run
        xt = np.zeros((128, width), dtype=np.float32)
        xt[:, 0:cols] = np.lib.stride_tricks.as_strided(
            x[base:], shape=(128, cols), strides=(run * 4, 4)
        )
        # weight of window t of partition p: global index i = i0 + run-1-t
        # (np.power, not exp(log*expo): ff == 0.0 needs 0^0 == 1)
        i0 = L - 1 - (base + run * p + (run - 1))
        expo = (i0[:, None] + (run - 1 - t)[None, :]).astype(np.float64)
        xt[:, cols : cols + run] = (np.power(ff64, expo) / 127.0).astype(
            np.float32
        )
        in_maps.append({"xt": xt})
    return in_maps


def combine_host(accs: list[np.ndarray], ff32: np.float32) -> np.ndarray:
    """accs: per-core [128,64] tiles, partial sums in column 0. f64 host sum."""
    ff64 = np.float64(ff32)
    total = np.float64(0.0)
    for c in range(NCORES):
        total += np.asarray(accs[c])[:, 0].astype(np.float64).sum()
    norm = (1.0 - ff64) / (1.0 - np.power(ff64, np.float64(L)))
    return np.asarray(np.float32(norm * total))


def kernel(past_returns, features, raw_forgetting_factor):
    x = np.ascontiguousarray(np.asarray(past_returns, dtype=np.float32))
    assert x.shape == (N,), x.shape
    raw = np.float64(np.asarray(raw_forgetting_factor).reshape(-1)[0])
    ff32 = np.float32(1.0 / (1.0 + np.exp(-raw)))

    run = plan_run(np.float64(ff32))
    nc = _get_nc(run)
    in_maps = make_in_maps(x, ff32, run)
    res = run_bass_kernel_spmd(nc, in_maps, list(range(NCORES)))
    accs = [res.results[c]["acc"] for c in range(NCORES)]
    return combine_host(accs, ff32)


# revision 8
# speedup vs baseline: 1.0102x; 1.0102x over previous
"""EWMA predictor (sliding-window variance, exponentially weighted sum) on 8 trn2 cores.

Math: for j in [0, L): window_j = x[j : j+128], weight ff^(L-1-j),
result = norm * sum_j ff^(L-1-j) * var(window_j, ddof=1),
norm = (1-ff)/(1-ff^L), ff = sigmoid(raw_forgetting_factor).

Sharding: windows split over 8 cores x 128 partitions; partition p of core c
owns `run` consecutive windows and loads the run+127 input elements covering
them (halo overlap) plus a per-window weight row
WT[p, t] = ff^(i0(c,p) + run-1-t) / 127 in the trailing columns.

Device program (per core):
  input:  SWDGE dma_gather; row indices built on-device (Pool iotas + DVE
          int32 and/add/cast, replicated in every 16-partition group since
          each Q7 core reads the wrapped idxs from its own group), the
          descriptors prepared and triggered early so the completion
          latency overlaps the whole compute chain.
  chain:  all on DVE: X2 = x^2; one fused [128,2,W] reduce seeds both
          sliding-sum recurrences; two tensor_tensor_scan ops produce the
          128-window sums s1/s2; d = s2 - s1^2/128 (= 127*var_unbiased);
          contrib[p] = sum_t WT[p,t]*d[p,t] (elementwise mult + reduce).
  output: SWDGE dma_scatter_add writes contrib[p] into acc[p,0] (stride
          256B rows; output DRAM buffers are zero-initialized by both the
          native runner and the bass2jax/PJRT path).
Host sums the 8x128 partials and applies norm in float64.
"""

import numpy as np

import concourse.bass as bass
import concourse.mybir as mybir
from concourse import library_config
from concourse.library_overlay import lower_extended_insts
from concourse.bass_utils import run_bass_kernel_spmd

L = 524288          # look-back windows
W = 128             # variance window length
N = L + W           # input length
NCORES = 8
WIN_PER_CORE = L // NCORES      # 65536
RUN = WIN_PER_CORE // 128       # 512 windows per partition (full computation)

_NC_CACHE = {}


def plan_run(ff64: float) -> int:
    """Windows-per-partition for the adaptive program.

    Weights ff^i are EXACTLY zero in fp32 (past subnormals) once
    i > 104/|ln ff|, so windows beyond that cannot affect any output bit.
    Keep a >=1024-window margin, round the 1024*run window count up to a
    power-of-two run, clamp to [8, 512]; run=512 is the exact full
    computation (all L windows).
    """
    lnff = np.log(np.float64(ff64))
    if not (lnff < -1e-9):
        return RUN
    k_needed = 104.0 / (-lnff)
    run_min = int(np.ceil((k_needed + 1024.0) / 1024.0))
    run = 8
    while run < run_min:
        run *= 2
    return min(run, RUN)


def geom(run: int) -> tuple[int, int]:
    cols = run + W - 1
    width = (cols + run + 63) // 64 * 64
    return cols, width


def build_nc(run: int = 8) -> bass.Bass:
    cols, width = geom(run)
    nc = bass.Bass(trn_type="TRN2")
    f32 = mybir.dt.float32
    i32 = mybir.dt.int32
    i16 = mybir.dt.int16
    A = mybir.AluOpType
    xt = nc.declare_dram_parameter("xt", [128, width], f32, isOutput=False)
    acc = nc.declare_dram_parameter("acc", [128, 64], f32, isOutput=True)

    with (
        nc.sbuf_tensor([128, 2 * width], f32) as XX,
        nc.sbuf_tensor([128, 2 * run], f32) as S,
        nc.sbuf_tensor([128, run], f32) as D,
        nc.sbuf_tensor([128, run], f32) as E,
        nc.sbuf_tensor([128, 1], f32) as CONTRIB,
        nc.sbuf_tensor([128, 8], i32) as C32,
        nc.sbuf_tensor([128, 1], i32) as P32,
        nc.sbuf_tensor([128, 8], i32) as V32,
        nc.sbuf_tensor([128, 8], i16) as IDX,
        nc.sbuf_tensor([128, 134], f32) as DLY,
        nc.sbuf_tensor([1, 128], f32) as PDLY,
        nc.semaphore() as gsem,    # gather-in completion (DMA sem)
        nc.semaphore() as scsem,   # scatter-out completion (DMA sem)
        nc.semaphore() as psem,    # pool setup progress
        nc.semaphore() as esem,    # gather trigger enqueued
        nc.semaphore() as vsem,    # DVE progress (idx build + chain)
        nc.Block() as block,
    ):
        X = XX[:, 0:width]
        X2 = XX[:, width : width + cols]
        WT = XX[:, cols : cols + run]
        S1 = S[:, 0:run]
        S2 = S[:, run : 2 * run]

        @block.gpsimd
        def _(gpsimd):
            # Gather row index ingredients: IDX[p, c] must be (p%16) + 16c in
            # every 16-partition group (each Q7 core reads the wrapped idxs
            # from its own group; the interpreter reads group 0, hardware
            # group 1). Pool emits the affine parts, DVE the bitwise ones
            # (walrus: int bitwise ops are DVE-only, int16 alu is DVE-only).
            gpsimd.iota(
                C32[:], pattern=[[16, 8]], base=0, channel_multiplier=0
            ).then_inc(psem, 1)
            gpsimd.iota(
                P32[:], pattern=[[0, 1]], base=0, channel_multiplier=1
            ).then_inc(psem, 1)
            gpsimd.memset(PDLY[:, 0:64], 0)
            gpsimd.memset(PDLY[:, 64:120], 0)
            gpsimd.wait_ge(psem, 2)
            gpsimd.wait_ge(vsem, 1)        # P32 &= 15 done on DVE
            gpsimd.tensor_tensor(
                V32[:], C32[:], P32[:].broadcast_to([128, 8]), op=A.add
            ).then_inc(psem, 1)
            gpsimd.wait_ge(psem, 3)
            gpsimd.tensor_copy(IDX[:], V32[:]).then_inc(psem, 1)
            gpsimd.load_library(library_config.mlp)
            gpsimd.wait_ge(psem, 4)
            gpsimd.dma_gather(
                X.unsqueeze(1), xt[:], IDX[:], num_idxs=128, num_idxs_reg=128,
                elem_size=width, prepare_only=True, sem=gsem,
            ).then_inc(psem, 1)
            gpsimd.wait_ge(psem, 5)
            gpsimd.trigger_dma(1)          # fire gather-in
            gpsimd.sem_inc(esem, 1)        # consumers may now wait on gsem
            gpsimd.dma_scatter_add(
                acc[0:128, 0:1], CONTRIB[:], IDX[:], num_idxs=128,
                num_idxs_reg=128, elem_size=1, elem_step=64,
                prepare_only=True, sem=scsem,
            ).then_inc(psem, 1)
            gpsimd.wait_ge(psem, 6)
            gpsimd.wait_ge(vsem, 9)        # contrib ready
            gpsimd.trigger_dma(1)          # fire scatter-out
            gpsimd.wait_ge(scsem, 16)      # real-HW: scatter landed before exit

        @block.vector
        def _(vector):
            # idx build step 1 (bitwise is DVE-only): P32 &= 15
            vector.wait_ge(psem, 2)
            vector.tensor_scalar(
                P32[:], P32[:], 15, None, op0=A.bitwise_and
            ).then_inc(vsem, 1)
            # self-delay sized to arrive at the waits just after the Pool
            # trigger enqueues the gather (eager sem pass); if the trigger is
            # later than this, the blocked wait wakes normally (+100ns).
            vector.memset(DLY[:], 0.0)

            # compute chain
            vector.wait_ge(esem, 1)
            vector.wait_ge(gsem, 16)
            vector.scalar_tensor_tensor(
                X2[:], X[:, 0:cols], 1.0, X[:, 0:cols], op0=A.mult, op1=A.mult
            ).then_inc(vsem, 1)
            vector.wait_ge(vsem, 2)
            # fused initial sums: S[:, 0] = sum x[0:W], S[:, run] = sum x2[0:W]
            vector.reduce_sum(
                S[:].rearrange("p (g r) -> p g r", g=2)[:, :, 0:1],
                XX[:].rearrange("p (g c) -> p g c", g=2)[:, :, 0:W],
                axis=mybir.AxisListType.X,
            ).then_inc(vsem, 1)
            vector.wait_ge(vsem, 3)
            # sliding-sum scans: s[t] = (x[t+W-1] + s[t-1]) - x[t-1]
            vector.tensor_tensor_scan(
                S1[:, 1:run], X[:, W:cols], X[:, 0 : run - 1],
                initial=S1[:, 0:1], op0=A.add, op1=A.subtract,
            ).then_inc(vsem, 1)
            vector.wait_ge(vsem, 4)
            vector.tensor_tensor_scan(
                S2[:, 1:run], X2[:, W:cols], XX[:, width : width + run - 1],
                initial=S2[:, 0:1], op0=A.add, op1=A.subtract,
            ).then_inc(vsem, 1)
            vector.wait_ge(vsem, 5)
            # d = s2 - s1^2/128  (D = (s1 * -1/128) * s1; D = D + s2)
            vector.scalar_tensor_tensor(
                D[:], S1[:], -1.0 / 128.0, S1[:], op0=A.mult, op1=A.mult
            ).then_inc(vsem, 1)
            vector.wait_ge(vsem, 6)
            vector.scalar_tensor_tensor(
                D[:], D[:], 1.0, S2[:], op0=A.mult, op1=A.add
            ).then_inc(vsem, 1)
            vector.wait_ge(vsem, 7)
            # contrib[p] = sum_t WT[p,t] * d[p,t]
            vector.scalar_tensor_tensor(
                E[:], D[:], 1.0, WT[:], op0=A.mult, op1=A.mult
            ).then_inc(vsem, 1)
            vector.wait_ge(vsem, 8)
            vector.reduce_sum(
                CONTRIB[:], E[:], axis=mybir.AxisListType.X
            ).then_inc(vsem, 1)

    lower_extended_insts(nc)  # encode ISA bytes for the NEFF compiler
    return nc


def _get_nc(run: int) -> bass.Bass:
    if run not in _NC_CACHE:
        _NC_CACHE[run] = build_nc(run)
    return _NC_CACHE[run]


def make_in_maps(
    x: np.ndarray, ff32: np.float32, run: int
) -> list[dict[str, np.ndarray]]:
    """Per-core input tiles covering the last 1024*run windows (all L windows
    when run=512); slot (c, p) owns windows starting at
    L - 1024*run + (c*128 + p)*run."""
    cols, width = geom(run)
    start0 = L - 1024 * run
    ff64 = np.float64(ff32)
    p = np.arange(128)
    t = np.arange(run)
    in_maps = []
    for c in range(NCORES):
        base = start0 + c * 128 * run
        xt = np.zeros((128, width), dtype=np.float32)
        xt[:, 0:cols] = np.lib.stride_tricks.as_strided(
            x[base:], shape=(128, cols), strides=(run * 4, 4)
        )
        # weight of window t of partition p: global index i = i0 + run-1-t
        # (np.power, not exp(log*expo): ff == 0.0 needs 0^0 == 1)
        i0 = L - 1 - (base + run * p + (run - 1))
        expo = (i0[:, None] + (run - 1 - t)[None, :]).astype(np.float64)
        xt[:, cols : cols + run] = (np.power(ff64, expo) / 127.0).astype(
            np.float32
        )
        in_maps.append({"xt": xt})
    return in_maps


def combine_host(accs: list[np.ndarray], ff32: np.float32) -> np.ndarray:
    """accs: per-core [128,64] tiles, partial sums in column 0. f64 host sum."""
    ff64 = np.float64(ff32)
    total = np.float64(0.0)
    for c in range(NCORES):
        total += np.asarray(accs[c])[:, 0].astype(np.float64).sum()
    norm = (1.0 - ff64) / (1.0 - np.power(ff64, np.float64(L)))
    return np.asarray(np.float32(norm * total))


def kernel(past_returns, features, raw_forgetting_factor):
    x = np.ascontiguousarray(np.asarray(past_returns, dtype=np.float32))
    assert x.shape == (N,), x.shape
    raw = np.float64(np.asarray(raw_forgetting_factor).reshape(-1)[0])
    ff32 = np.float32(1.0 / (1.0 + np.exp(-raw)))

    run = plan_run(np.float64(ff32))
    nc = _get_nc(run)
    in_maps = make_in_maps(x, ff32, run)
    res = run_bass_kernel_spmd(nc, in_maps, list(range(NCORES)))
    accs = [res.results[c]["acc"] for c in range(NCORES)]
    return combine_host(accs, ff32)


# revision 9
# speedup vs baseline: 1.0163x; 1.0060x over previous
"""EWMA predictor (sliding-window variance, exponentially weighted sum) on 8 trn2 cores.

Math: for j in [0, L): window_j = x[j : j+128], weight ff^(L-1-j),
result = norm * sum_j ff^(L-1-j) * var(window_j, ddof=1),
norm = (1-ff)/(1-ff^L), ff = sigmoid(raw_forgetting_factor).

Sharding: windows split over 8 cores x 128 partitions; partition p of core c
owns `run` consecutive windows and loads the run+127 input elements covering
them (halo overlap) plus a per-window weight row
WT[p, t] = ff^(i0(c,p) + run-1-t) / 127 in the trailing columns.

Device program (per core):
  input:  SWDGE dma_gather; row indices built on-device (Pool iotas + DVE
          int32 and/add/cast, replicated in every 16-partition group since
          each Q7 core reads the wrapped idxs from its own group), the
          descriptors prepared and triggered early so the completion
          latency overlaps the whole compute chain.
  chain:  all on DVE: X2 = x^2; one fused [128,2,W] reduce seeds both
          sliding-sum recurrences; two tensor_tensor_scan ops produce the
          128-window sums s1/s2; d = s2 - s1^2/128 (= 127*var_unbiased);
          contrib[p] = sum_t WT[p,t]*d[p,t] (elementwise mult + reduce).
  output: SWDGE dma_scatter_add writes contrib[p] into acc[p,0] (stride
          256B rows; output DRAM buffers are zero-initialized by both the
          native runner and the bass2jax/PJRT path).
Host sums the 8x128 partials and applies norm in float64.
"""

import numpy as np

import concourse.bass as bass
import concourse.mybir as mybir
from concourse import library_config
from concourse.library_overlay import lower_extended_insts
from concourse.bass_utils import run_bass_kernel_spmd

L = 524288          # look-back windows
W = 128             # variance window length
N = L + W           # input length
NCORES = 8
WIN_PER_CORE = L // NCORES      # 65536
RUN = WIN_PER_CORE // 128       # 512 windows per partition (full computation)

_NC_CACHE = {}


def plan_run(ff64: float) -> int:
    """Windows-per-partition for the adaptive program.

    Weights ff^i are EXACTLY zero in fp32 (past subnormals) once
    i > 104/|ln ff|, so windows beyond that cannot affect any output bit.
    Keep a >=1024-window margin, round the 1024*run window count up to a
    power-of-two run, clamp to [8, 512]; run=512 is the exact full
    computation (all L windows).
    """
    lnff = np.log(np.float64(ff64))
    if not (lnff < -1e-9):
        return RUN
    k_needed = 104.0 / (-lnff)
    run_min = int(np.ceil((k_needed + 1024.0) / 1024.0))
    run = 8
    while run < run_min:
        run *= 2
    return min(run, RUN)


def geom(run: int) -> tuple[int, int]:
    cols = run + W - 1
    width = (cols + run + 63) // 64 * 64
    return cols, width


def build_nc(run: int = 8) -> bass.Bass:
    cols, width = geom(run)
    nc = bass.Bass(trn_type="TRN2")
    f32 = mybir.dt.float32
    i32 = mybir.dt.int32
    i16 = mybir.dt.int16
    A = mybir.AluOpType
    xt = nc.declare_dram_parameter("xt", [128, width], f32, isOutput=False)
    acc = nc.declare_dram_parameter("acc", [128, 64], f32, isOutput=True)

    with (
        nc.sbuf_tensor([128, 2 * width], f32) as XX,
        nc.sbuf_tensor([128, 2 * run], f32) as S,
        nc.sbuf_tensor([128, run], f32) as D,
        nc.sbuf_tensor([128, run], f32) as E,
        nc.sbuf_tensor([128, 1], f32) as CONTRIB,
        nc.sbuf_tensor([128, 8], i32) as C32,
        nc.sbuf_tensor([128, 1], i32) as P32,
        nc.sbuf_tensor([128, 8], i32) as V32,
        nc.sbuf_tensor([128, 8], i16) as IDX,
        nc.sbuf_tensor([128, 124], f32) as DLY,
        nc.sbuf_tensor([1, 128], f32) as PDLY,
        nc.semaphore() as gsem,    # gather-in completion (DMA sem)
        nc.semaphore() as scsem,   # scatter-out completion (DMA sem)
        nc.semaphore() as psem,    # pool setup progress
        nc.semaphore() as esem,    # gather trigger enqueued
        nc.semaphore() as vsem,    # DVE progress (idx build + chain)
        nc.Block() as block,
    ):
        X = XX[:, 0:width]
        X2 = XX[:, width : width + cols]
        WT = XX[:, cols : cols + run]
        S1 = S[:, 0:run]
        S2 = S[:, run : 2 * run]

        @block.gpsimd
        def _(gpsimd):
            # Gather row index ingredients: IDX[p, c] must be (p%16) + 16c in
            # every 16-partition group (each Q7 core reads the wrapped idxs
            # from its own group; the interpreter reads group 0, hardware
            # group 1). Pool emits the affine parts, DVE the bitwise ones
            # (walrus: int bitwise ops are DVE-only, int16 alu is DVE-only).
            gpsimd.iota(
                C32[:], pattern=[[16, 8]], base=0, channel_multiplier=0
            ).then_inc(psem, 1)
            gpsimd.iota(
                P32[:], pattern=[[0, 1]], base=0, channel_multiplier=1
            ).then_inc(psem, 1)
            gpsimd.memset(PDLY[:, 0:64], 0)
            gpsimd.memset(PDLY[:, 64:120], 0)
            gpsimd.wait_ge(psem, 2)
            gpsimd.wait_ge(vsem, 1)        # P32 &= 15 done on DVE
            gpsimd.tensor_tensor(
                V32[:], C32[:], P32[:].broadcast_to([128, 8]), op=A.add
            ).then_inc(psem, 1)
            gpsimd.wait_ge(psem, 3)
            gpsimd.tensor_copy(IDX[:], V32[:]).then_inc(psem, 1)
            gpsimd.load_library(library_config.mlp)
            gpsimd.wait_ge(psem, 4)
            gpsimd.dma_gather(
                X.unsqueeze(1), xt[:], IDX[:], num_idxs=128, num_idxs_reg=128,
                elem_size=width, prepare_only=True, sem=gsem,
            ).then_inc(psem, 1)
            gpsimd.wait_ge(psem, 5)
            gpsimd.trigger_dma(1)          # fire gather-in
            gpsimd.sem_inc(esem, 1)        # consumers may now wait on gsem
            gpsimd.dma_scatter_add(
                acc[0:128, 0:1], CONTRIB[:], IDX[:], num_idxs=128,
                num_idxs_reg=128, elem_size=1, elem_step=64,
                prepare_only=True, sem=scsem,
            ).then_inc(psem, 1)
            gpsimd.wait_ge(psem, 6)
            gpsimd.wait_ge(vsem, 9)        # contrib ready
            gpsimd.trigger_dma(1)          # fire scatter-out
            gpsimd.wait_ge(scsem, 16)      # real-HW: scatter landed before exit

        @block.vector
        def _(vector):
            # idx build step 1 (bitwise is DVE-only): P32 &= 15
            vector.wait_ge(psem, 2)
            vector.tensor_scalar(
                P32[:], P32[:], 15, None, op0=A.bitwise_and
            ).then_inc(vsem, 1)
            # self-delay sized to arrive at the waits just after the Pool
            # trigger enqueues the gather (eager sem pass); if the trigger is
            # later than this, the blocked wait wakes normally (+100ns).
            vector.memset(DLY[:], 0.0)

            # compute chain
            vector.wait_ge(esem, 1)
            vector.wait_ge(gsem, 16)
            vector.scalar_tensor_tensor(
                X2[:], X[:, 0:cols], 1.0, X[:, 0:cols], op0=A.mult, op1=A.mult
            ).then_inc(vsem, 1)
            vector.wait_ge(vsem, 2)
            # fused initial sums: S[:, 0] = sum x[0:W], S[:, run] = sum x2[0:W]
            vector.reduce_sum(
                S[:].rearrange("p (g r) -> p g r", g=2)[:, :, 0:1],
                XX[:].rearrange("p (g c) -> p g c", g=2)[:, :, 0:W],
                axis=mybir.AxisListType.X,
            ).then_inc(vsem, 1)
            vector.wait_ge(vsem, 3)
            # sliding-sum scans: s[t] = (x[t+W-1] + s[t-1]) - x[t-1]
            vector.tensor_tensor_scan(
                S1[:, 1:run], X[:, W:cols], X[:, 0 : run - 1],
                initial=S1[:, 0:1], op0=A.add, op1=A.subtract,
            ).then_inc(vsem, 1)
            vector.wait_ge(vsem, 4)
            vector.tensor_tensor_scan(
                S2[:, 1:run], X2[:, W:cols], XX[:, width : width + run - 1],
                initial=S2[:, 0:1], op0=A.add, op1=A.subtract,
            ).then_inc(vsem, 1)
            vector.wait_ge(vsem, 5)
            # d = s2 - s1^2/128  (D = (s1 * -1/128) * s1; D = D + s2)
            vector.scalar_tensor_tensor(
                D[:], S1[:], -1.0 / 128.0, S1[:], op0=A.mult, op1=A.mult
            ).then_inc(vsem, 1)
            vector.wait_ge(vsem, 6)
            vector.scalar_tensor_tensor(
                D[:], D[:], 1.0, S2[:], op0=A.mult, op1=A.add
            ).then_inc(vsem, 1)
            vector.wait_ge(vsem, 7)
            # contrib[p] = sum_t WT[p,t] * d[p,t]
            vector.scalar_tensor_tensor(
                E[:], D[:], 1.0, WT[:], op0=A.mult, op1=A.mult
            ).then_inc(vsem, 1)
            vector.wait_ge(vsem, 8)
            vector.reduce_sum(
                CONTRIB[:], E[:], axis=mybir.AxisListType.X
            ).then_inc(vsem, 1)

    lower_extended_insts(nc)  # encode ISA bytes for the NEFF compiler
    return nc


def _get_nc(run: int) -> bass.Bass:
    if run not in _NC_CACHE:
        _NC_CACHE[run] = build_nc(run)
    return _NC_CACHE[run]


def make_in_maps(
    x: np.ndarray, ff32: np.float32, run: int
) -> list[dict[str, np.ndarray]]:
    """Per-core input tiles covering the last 1024*run windows (all L windows
    when run=512); slot (c, p) owns windows starting at
    L - 1024*run + (c*128 + p)*run."""
    cols, width = geom(run)
    start0 = L - 1024 * run
    ff64 = np.float64(ff32)
    p = np.arange(128)
    t = np.arange(run)
    in_maps = []
    for c in range(NCORES):
        base = start0 + c * 128 * run
        xt = np.zeros((128, width), dtype=np.float32)
        xt[:, 0:cols] = np.lib.stride_tricks.as_strided(
            x[base:], shape=(128, cols), strides=(run * 4, 4)
        )
        # weight of window t of partition p: global index i = i0 + run-1-t
        # (np.power, not exp(log*expo): ff == 0.0 needs 0^0 == 1)
        i0 = L - 1 - (base + run * p + (run - 1))
        expo = (i0[:, None] + (run - 1 - t)[None, :]).astype(np.float64)
        xt[:, cols : cols + run] = (np.power(ff64, expo) / 127.0).astype(
            np.float32
        )
        in_maps.append({"xt": xt})
    return in_maps


def combine_host(accs: list[np.ndarray], ff32: np.float32) -> np.ndarray:
    """accs: per-core [128,64] tiles, partial sums in column 0. f64 host sum."""
    ff64 = np.float64(ff32)
    total = np.float64(0.0)
    for c in range(NCORES):
        total += np.asarray(accs[c])[:, 0].astype(np.float64).sum()
    norm = (1.0 - ff64) / (1.0 - np.power(ff64, np.float64(L)))
    return np.asarray(np.float32(norm * total))


def kernel(past_returns, features, raw_forgetting_factor):
    x = np.ascontiguousarray(np.asarray(past_returns, dtype=np.float32))
    assert x.shape == (N,), x.shape
    raw = np.float64(np.asarray(raw_forgetting_factor).reshape(-1)[0])
    ff32 = np.float32(1.0 / (1.0 + np.exp(-raw)))

    run = plan_run(np.float64(ff32))
    nc = _get_nc(run)
    in_maps = make_in_maps(x, ff32, run)
    res = run_bass_kernel_spmd(nc, in_maps, list(range(NCORES)))
    accs = [res.results[c]["acc"] for c in range(NCORES)]
    return combine_host(accs, ff32)


# revision 10
# speedup vs baseline: 1.0305x; 1.0140x over previous
"""EWMA predictor (sliding-window variance, exponentially weighted sum) on 8 trn2 cores.

Math: for j in [0, L): window_j = x[j : j+128], weight ff^(L-1-j),
result = norm * sum_j ff^(L-1-j) * var(window_j, ddof=1),
norm = (1-ff)/(1-ff^L), ff = sigmoid(raw_forgetting_factor).

Sharding: windows split over 8 cores x 128 partitions; partition p of core c
owns `run` consecutive windows and loads the run+127 input elements covering
them (halo overlap) plus a per-window weight row
WT[p, t] = ff^(i0(c,p) + run-1-t) / 127 in the trailing columns.

Device program (per core):
  input:  SWDGE dma_gather; row indices built on-device (Pool iotas + DVE
          int32 and/add/cast, replicated in every 16-partition group since
          each Q7 core reads the wrapped idxs from its own group), the
          descriptors prepared and triggered early so the completion
          latency overlaps the whole compute chain.
  chain:  all on DVE: X2 = x^2; one fused [128,2,W] reduce seeds both
          sliding-sum recurrences; two tensor_tensor_scan ops produce the
          128-window sums s1/s2; d = s2 - s1^2/128 (= 127*var_unbiased);
          contrib[p] = sum_t WT[p,t]*d[p,t] (elementwise mult + reduce).
  output: SWDGE dma_scatter_add writes contrib[p] into acc[p,0] (stride
          256B rows; output DRAM buffers are zero-initialized by both the
          native runner and the bass2jax/PJRT path).
Host sums the 8x128 partials and applies norm in float64.
"""

import numpy as np

import concourse.bass as bass
import concourse.mybir as mybir
from concourse import library_config
from concourse.library_overlay import lower_extended_insts
from concourse.bass_utils import run_bass_kernel_spmd

L = 524288          # look-back windows
W = 128             # variance window length
N = L + W           # input length
NCORES = 8
WIN_PER_CORE = L // NCORES      # 65536
RUN = WIN_PER_CORE // 128       # 512 windows per partition (full computation)

_NC_CACHE = {}


def plan_run(ff64: float) -> int:
    """Windows-per-partition for the adaptive program.

    Weights ff^i are EXACTLY zero in fp32 (past subnormals) once
    i > 104/|ln ff|, so windows beyond that cannot affect any output bit.
    Keep a >=1024-window margin, round the 1024*run window count up to a
    power-of-two run, clamp to [8, 512]; run=512 is the exact full
    computation (all L windows).
    """
    lnff = np.log(np.float64(ff64))
    if not (lnff < -1e-9):
        return RUN
    k_needed = 104.0 / (-lnff)
    run_min = int(np.ceil((k_needed + 1024.0) / 1024.0))
    run = 8
    while run < run_min:
        run *= 2
    return min(run, RUN)


def geom(run: int) -> tuple[int, int]:
    cols = run + W - 1
    width = (cols + run + 63) // 64 * 64
    return cols, width


def build_nc(run: int = 8) -> bass.Bass:
    cols, width = geom(run)
    nc = bass.Bass(trn_type="TRN2")
    f32 = mybir.dt.float32
    i32 = mybir.dt.int32
    i16 = mybir.dt.int16
    A = mybir.AluOpType
    xt = nc.declare_dram_parameter("xt", [128, width], f32, isOutput=False)
    acc = nc.declare_dram_parameter("acc", [128, 64], f32, isOutput=True)

    with (
        nc.sbuf_tensor([128, 2 * width], f32) as XX,
        nc.sbuf_tensor([128, 2 * run], f32) as S,
        nc.sbuf_tensor([128, run], f32) as D,
        nc.sbuf_tensor([128, run], f32) as E,
        nc.sbuf_tensor([128, 1], f32) as CONTRIB,
        nc.sbuf_tensor([128, 8], i32) as C32,
        nc.sbuf_tensor([128, 1], i32) as P32,
        nc.sbuf_tensor([128, 8], i32) as V32,
        nc.sbuf_tensor([128, 8], i16) as IDX,
        nc.sbuf_tensor([128, 124], f32) as DLY,
        nc.sbuf_tensor([1, 1280], f32) as PDLY,
        nc.semaphore() as gsem,    # gather-in completion (DMA sem)
        nc.semaphore() as scsem,   # scatter-out completion (DMA sem)
        nc.semaphore() as psem,    # pool setup progress
        nc.semaphore() as esem,    # gather trigger enqueued
        nc.semaphore() as vsem,    # DVE progress (idx build + chain)
        nc.Block() as block,
    ):
        X = XX[:, 0:width]
        X2 = XX[:, width : width + cols]
        WT = XX[:, cols : cols + run]
        S1 = S[:, 0:run]
        S2 = S[:, run : 2 * run]

        @block.gpsimd
        def _(gpsimd):
            # Gather row index ingredients: IDX[p, c] must be (p%16) + 16c in
            # every 16-partition group (each Q7 core reads the wrapped idxs
            # from its own group; the interpreter reads group 0, hardware
            # group 1). Pool emits the affine parts, DVE the bitwise ones
            # (walrus: int bitwise ops are DVE-only, int16 alu is DVE-only).
            gpsimd.iota(
                C32[:], pattern=[[16, 8]], base=0, channel_multiplier=0
            ).then_inc(psem, 1)
            gpsimd.iota(
                P32[:], pattern=[[0, 1]], base=0, channel_multiplier=1
            ).then_inc(psem, 1)
            gpsimd.memset(PDLY[:, 0:64], 0)
            gpsimd.memset(PDLY[:, 64:120], 0)
            gpsimd.wait_ge(psem, 2)
            gpsimd.wait_ge(vsem, 1)        # P32 &= 15 done on DVE
            gpsimd.tensor_tensor(
                V32[:], C32[:], P32[:].broadcast_to([128, 8]), op=A.add
            ).then_inc(psem, 1)
            gpsimd.wait_ge(psem, 3)
            gpsimd.tensor_copy(IDX[:], V32[:]).then_inc(psem, 1)
            gpsimd.load_library(library_config.mlp)
            gpsimd.wait_ge(psem, 4)
            gpsimd.dma_gather(
                X.unsqueeze(1), xt[:], IDX[:], num_idxs=128, num_idxs_reg=128,
                elem_size=width, prepare_only=True, sem=gsem,
            ).then_inc(psem, 1)
            gpsimd.wait_ge(psem, 5)
            gpsimd.trigger_dma(1)          # fire gather-in
            gpsimd.sem_inc(esem, 1)        # consumers may now wait on gsem
            gpsimd.dma_scatter_add(
                acc[0:128, 0:1], CONTRIB[:], IDX[:], num_idxs=128,
                num_idxs_reg=128, elem_size=1, elem_step=64,
                prepare_only=True, sem=scsem,
            ).then_inc(psem, 1)
            gpsimd.wait_ge(psem, 6)
            # self-delay: arrive at the vsem wait just after the run=8 chain's
            # last enqueue (eager pass); longer chains fall back to the normal
            # blocked wake (+31ns), shorter configs never occur.
            gpsimd.memset(PDLY[:, 128:1268], 0)
            gpsimd.wait_ge(vsem, 9)        # contrib ready
            gpsimd.trigger_dma(1)          # fire scatter-out
            gpsimd.wait_ge(scsem, 16)      # real-HW: scatter landed before exit

        @block.vector
        def _(vector):
            # idx build step 1 (bitwise is DVE-only): P32 &= 15
            vector.wait_ge(psem, 2)
            vector.tensor_scalar(
                P32[:], P32[:], 15, None, op0=A.bitwise_and
            ).then_inc(vsem, 1)
            # self-delay sized to arrive at the waits just after the Pool
            # trigger enqueues the gather (eager sem pass); if the trigger is
            # later than this, the blocked wait wakes normally (+100ns).
            vector.memset(DLY[:], 0.0)

            # compute chain
            vector.wait_ge(esem, 1)
            vector.wait_ge(gsem, 16)
            vector.scalar_tensor_tensor(
                X2[:], X[:, 0:cols], 1.0, X[:, 0:cols], op0=A.mult, op1=A.mult
            ).then_inc(vsem, 1)
            vector.wait_ge(vsem, 2)
            # fused initial sums: S[:, 0] = sum x[0:W], S[:, run] = sum x2[0:W]
            vector.reduce_sum(
                S[:].rearrange("p (g r) -> p g r", g=2)[:, :, 0:1],
                XX[:].rearrange("p (g c) -> p g c", g=2)[:, :, 0:W],
                axis=mybir.AxisListType.X,
            ).then_inc(vsem, 1)
            vector.wait_ge(vsem, 3)
            # sliding-sum scans: s[t] = (x[t+W-1] + s[t-1]) - x[t-1]
            vector.tensor_tensor_scan(
                S1[:, 1:run], X[:, W:cols], X[:, 0 : run - 1],
                initial=S1[:, 0:1], op0=A.add, op1=A.subtract,
            ).then_inc(vsem, 1)
            vector.wait_ge(vsem, 4)
            vector.tensor_tensor_scan(
                S2[:, 1:run], X2[:, W:cols], XX[:, width : width + run - 1],
                initial=S2[:, 0:1], op0=A.add, op1=A.subtract,
            ).then_inc(vsem, 1)
            vector.wait_ge(vsem, 5)
            # d = s2 - s1^2/128  (D = (s1 * -1/128) * s1; D = D + s2)
            vector.scalar_tensor_tensor(
                D[:], S1[:], -1.0 / 128.0, S1[:], op0=A.mult, op1=A.mult
            ).then_inc(vsem, 1)
            vector.wait_ge(vsem, 6)
            vector.scalar_tensor_tensor(
                D[:], D[:], 1.0, S2[:], op0=A.mult, op1=A.add
            ).then_inc(vsem, 1)
            vector.wait_ge(vsem, 7)
            # contrib[p] = sum_t WT[p,t] * d[p,t]
            vector.scalar_tensor_tensor(
                E[:], D[:], 1.0, WT[:], op0=A.mult, op1=A.mult
            ).then_inc(vsem, 1)
            vector.wait_ge(vsem, 8)
            vector.reduce_sum(
                CONTRIB[:], E[:], axis=mybir.AxisListType.X
            ).then_inc(vsem, 1)

    lower_extended_insts(nc)  # encode ISA bytes for the NEFF compiler
    return nc


def _get_nc(run: int) -> bass.Bass:
    if run not in _NC_CACHE:
        _NC_CACHE[run] = build_nc(run)
    return _NC_CACHE[run]


def make_in_maps(
    x: np.ndarray, ff32: np.float32, run: int
) -> list[dict[str, np.ndarray]]:
    """Per-core input tiles covering the last 1024*run windows (all L windows
    when run=512); slot (c, p) owns windows starting at
    L - 1024*run + (c*128 + p)*run."""
    cols, width = geom(run)
    start0 = L - 1024 * run
    ff64 = np.float64(ff32)
    p = np.arange(128)
    t = np.arange(run)
    in_maps = []
    for c in range(NCORES):
        base = start0 + c * 128 * run
        xt = np.zeros((128, width), dtype=np.float32)
        xt[:, 0:cols] = np.lib.stride_tricks.as_strided(
            x[base:], shape=(128, cols), strides=(run * 4, 4)
        )
        # weight of window t of partition p: global index i = i0 + run-1-t
        # (np.power, not exp(log*expo): ff == 0.0 needs 0^0 == 1)
        i0 = L - 1 - (base + run * p + (run - 1))
        expo = (i0[:, None] + (run - 1 - t)[None, :]).astype(np.float64)
        xt[:, cols : cols + run] = (np.power(ff64, expo) / 127.0).astype(
            np.float32
        )
        in_maps.append({"xt": xt})
    return in_maps


def combine_host(accs: list[np.ndarray], ff32: np.float32) -> np.ndarray:
    """accs: per-core [128,64] tiles, partial sums in column 0. f64 host sum."""
    ff64 = np.float64(ff32)
    total = np.float64(0.0)
    for c in range(NCORES):
        total += np.asarray(accs[c])[:, 0].astype(np.float64).sum()
    norm = (1.0 - ff64) / (1.0 - np.power(ff64, np.float64(L)))
    return np.asarray(np.float32(norm * total))


def kernel(past_returns, features, raw_forgetting_factor):
    x = np.ascontiguousarray(np.asarray(past_returns, dtype=np.float32))
    assert x.shape == (N,), x.shape
    raw = np.float64(np.asarray(raw_forgetting_factor).reshape(-1)[0])
    ff32 = np.float32(1.0 / (1.0 + np.exp(-raw)))

    run = plan_run(np.float64(ff32))
    nc = _get_nc(run)
    in_maps = make_in_maps(x, ff32, run)
    res = run_bass_kernel_spmd(nc, in_maps, list(range(NCORES)))
    accs = [res.results[c]["acc"] for c in range(NCORES)]
    return combine_host(accs, ff32)


# revision 11
# speedup vs baseline: 1.0330x; 1.0024x over previous
"""EWMA predictor (sliding-window variance, exponentially weighted sum) on 8 trn2 cores.

Math: for j in [0, L): window_j = x[j : j+128], weight ff^(L-1-j),
result = norm * sum_j ff^(L-1-j) * var(window_j, ddof=1),
norm = (1-ff)/(1-ff^L), ff = sigmoid(raw_forgetting_factor).

Sharding: windows split over 8 cores x 128 partitions; partition p of core c
owns `run` consecutive windows and loads the run+127 input elements covering
them (halo overlap) plus a per-window weight row
WT[p, t] = ff^(i0(c,p) + run-1-t) / 127 in the trailing columns.

Device program (per core):
  input:  SWDGE dma_gather; row indices built on-device (Pool iotas + DVE
          int32 and/add/cast, replicated in every 16-partition group since
          each Q7 core reads the wrapped idxs from its own group), the
          descriptors prepared and triggered early so the completion
          latency overlaps the whole compute chain.
  chain:  all on DVE: X2 = x^2; one fused [128,2,W] reduce seeds both
          sliding-sum recurrences; two tensor_tensor_scan ops produce the
          128-window sums s1/s2; d = s2 - s1^2/128 (= 127*var_unbiased);
          contrib[p] = sum_t WT[p,t]*d[p,t] (elementwise mult + reduce).
  output: SWDGE dma_scatter_add writes contrib[p] into acc[p,0] (stride
          256B rows; output DRAM buffers are zero-initialized by both the
          native runner and the bass2jax/PJRT path).
Host sums the 8x128 partials and applies norm in float64.
"""

import numpy as np

import concourse.bass as bass
import concourse.mybir as mybir
from concourse import library_config
from concourse.library_overlay import lower_extended_insts
from concourse.bass_utils import run_bass_kernel_spmd

L = 524288          # look-back windows
W = 128             # variance window length
N = L + W           # input length
NCORES = 8
WIN_PER_CORE = L // NCORES      # 65536
RUN = WIN_PER_CORE // 128       # 512 windows per partition (full computation)

_NC_CACHE = {}


def plan_run(ff64: float) -> int:
    """Windows-per-partition for the adaptive program.

    Weights ff^i are EXACTLY zero in fp32 (past subnormals) once
    i > 104/|ln ff|, so windows beyond that cannot affect any output bit.
    Keep a >=1024-window margin, round the 1024*run window count up to a
    power-of-two run, clamp to [8, 512]; run=512 is the exact full
    computation (all L windows).
    """
    lnff = np.log(np.float64(ff64))
    if not (lnff < -1e-9):
        return RUN
    k_needed = 104.0 / (-lnff)
    run_min = int(np.ceil((k_needed + 1024.0) / 1024.0))
    run = 8
    while run < run_min:
        run *= 2
    return min(run, RUN)


def geom(run: int) -> tuple[int, int]:
    cols = run + W - 1
    width = (cols + run + 63) // 64 * 64
    return cols, width


def build_nc(run: int = 8) -> bass.Bass:
    cols, width = geom(run)
    nc = bass.Bass(trn_type="TRN2")
    f32 = mybir.dt.float32
    i32 = mybir.dt.int32
    i16 = mybir.dt.int16
    A = mybir.AluOpType
    xt = nc.declare_dram_parameter("xt", [128, width], f32, isOutput=False)
    acc = nc.declare_dram_parameter("acc", [128, 64], f32, isOutput=True)

    with (
        nc.sbuf_tensor([128, 2 * width], f32) as XX,
        nc.sbuf_tensor([128, 2 * run], f32) as S,
        nc.sbuf_tensor([128, run], f32) as D,
        nc.sbuf_tensor([128, run], f32) as E,
        nc.sbuf_tensor([128, 1], f32) as CONTRIB,
        nc.sbuf_tensor([128, 8], i32) as C32,
        nc.sbuf_tensor([128, 1], i32) as P32,
        nc.sbuf_tensor([128, 8], i32) as V32,
        nc.sbuf_tensor([128, 8], i16) as IDX,
        nc.sbuf_tensor([128, 120], f32) as DLY,
        nc.sbuf_tensor([1, 1280], f32) as PDLY,
        nc.semaphore() as gsem,    # gather-in completion (DMA sem)
        nc.semaphore() as scsem,   # scatter-out completion (DMA sem)
        nc.semaphore() as psem,    # pool setup progress
        nc.semaphore() as esem,    # gather trigger enqueued
        nc.semaphore() as vsem,    # DVE progress (idx build + chain)
        nc.Block() as block,
    ):
        X = XX[:, 0:width]
        X2 = XX[:, width : width + cols]
        WT = XX[:, cols : cols + run]
        S1 = S[:, 0:run]
        S2 = S[:, run : 2 * run]

        @block.gpsimd
        def _(gpsimd):
            # Gather row index ingredients: IDX[p, c] must be (p%16) + 16c in
            # every 16-partition group (each Q7 core reads the wrapped idxs
            # from its own group; the interpreter reads group 0, hardware
            # group 1). Pool emits the affine parts, DVE the bitwise ones
            # (walrus: int bitwise ops are DVE-only, int16 alu is DVE-only).
            gpsimd.iota(
                C32[:], pattern=[[16, 8]], base=0, channel_multiplier=0
            ).then_inc(psem, 1)
            gpsimd.iota(
                P32[:], pattern=[[0, 1]], base=0, channel_multiplier=1
            ).then_inc(psem, 1)
            gpsimd.memset(PDLY[:, 0:64], 0)
            gpsimd.memset(PDLY[:, 64:116], 0)
            gpsimd.wait_ge(psem, 2)
            gpsimd.wait_ge(vsem, 1)        # P32 &= 15 done on DVE
            gpsimd.tensor_tensor(
                V32[:], C32[:], P32[:].broadcast_to([128, 8]), op=A.add
            ).then_inc(psem, 1)
            gpsimd.wait_ge(psem, 3)
            gpsimd.tensor_copy(IDX[:], V32[:]).then_inc(psem, 1)
            gpsimd.load_library(library_config.mlp)
            gpsimd.wait_ge(psem, 4)
            gpsimd.dma_gather(
                X.unsqueeze(1), xt[:], IDX[:], num_idxs=128, num_idxs_reg=128,
                elem_size=width, prepare_only=True, sem=gsem,
            ).then_inc(psem, 1)
            gpsimd.wait_ge(psem, 5)
            gpsimd.trigger_dma(1)          # fire gather-in
            gpsimd.sem_inc(esem, 1)        # consumers may now wait on gsem
            gpsimd.dma_scatter_add(
                acc[0:128, 0:1], CONTRIB[:], IDX[:], num_idxs=128,
                num_idxs_reg=128, elem_size=1, elem_step=64,
                prepare_only=True, sem=scsem,
            ).then_inc(psem, 1)
            gpsimd.wait_ge(psem, 6)
            # self-delay: arrive at the vsem wait just after the run=8 chain's
            # last enqueue (eager pass); longer chains fall back to the normal
            # blocked wake (+31ns), shorter configs never occur.
            gpsimd.memset(PDLY[:, 128:1268], 0)
            gpsimd.wait_ge(vsem, 9)        # contrib ready
            gpsimd.trigger_dma(1)          # fire scatter-out
            gpsimd.wait_ge(scsem, 16)      # real-HW: scatter landed before exit

        @block.vector
        def _(vector):
            # idx build step 1 (bitwise is DVE-only): P32 &= 15
            vector.wait_ge(psem, 2)
            vector.tensor_scalar(
                P32[:], P32[:], 15, None, op0=A.bitwise_and
            ).then_inc(vsem, 1)
            # self-delay sized to arrive at the waits just after the Pool
            # trigger enqueues the gather (eager sem pass); if the trigger is
            # later than this, the blocked wait wakes normally (+100ns).
            vector.memset(DLY[:], 0.0)

            # compute chain
            vector.wait_ge(esem, 1)
            vector.wait_ge(gsem, 16)
            vector.scalar_tensor_tensor(
                X2[:], X[:, 0:cols], 1.0, X[:, 0:cols], op0=A.mult, op1=A.mult
            ).then_inc(vsem, 1)
            vector.wait_ge(vsem, 2)
            # fused initial sums: S[:, 0] = sum x[0:W], S[:, run] = sum x2[0:W]
            vector.reduce_sum(
                S[:].rearrange("p (g r) -> p g r", g=2)[:, :, 0:1],
                XX[:].rearrange("p (g c) -> p g c", g=2)[:, :, 0:W],
                axis=mybir.AxisListType.X,
            ).then_inc(vsem, 1)
            vector.wait_ge(vsem, 3)
            # sliding-sum scans: s[t] = (x[t+W-1] + s[t-1]) - x[t-1]
            vector.tensor_tensor_scan(
                S1[:, 1:run], X[:, W:cols], X[:, 0 : run - 1],
                initial=S1[:, 0:1], op0=A.add, op1=A.subtract,
            ).then_inc(vsem, 1)
            vector.wait_ge(vsem, 4)
            vector.tensor_tensor_scan(
                S2[:, 1:run], X2[:, W:cols], XX[:, width : width + run - 1],
                initial=S2[:, 0:1], op0=A.add, op1=A.subtract,
            ).then_inc(vsem, 1)
            vector.wait_ge(vsem, 5)
            # d = s2 - s1^2/128  (D = (s1 * -1/128) * s1; D = D + s2)
            vector.scalar_tensor_tensor(
                D[:], S1[:], -1.0 / 128.0, S1[:], op0=A.mult, op1=A.mult
            ).then_inc(vsem, 1)
            vector.wait_ge(vsem, 6)
            vector.scalar_tensor_tensor(
                D[:], D[:], 1.0, S2[:], op0=A.mult, op1=A.add
            ).then_inc(vsem, 1)
            vector.wait_ge(vsem, 7)
            # contrib[p] = sum_t WT[p,t] * d[p,t]
            vector.scalar_tensor_tensor(
                E[:], D[:], 1.0, WT[:], op0=A.mult, op1=A.mult
            ).then_inc(vsem, 1)
            vector.wait_ge(vsem, 8)
            vector.reduce_sum(
                CONTRIB[:], E[:], axis=mybir.AxisListType.X
            ).then_inc(vsem, 1)

    lower_extended_insts(nc)  # encode ISA bytes for the NEFF compiler
    return nc


def _get_nc(run: int) -> bass.Bass:
    if run not in _NC_CACHE:
        _NC_CACHE[run] = build_nc(run)
    return _NC_CACHE[run]


def make_in_maps(
    x: np.ndarray, ff32: np.float32, run: int
) -> list[dict[str, np.ndarray]]:
    """Per-core input tiles covering the last 1024*run windows (all L windows
    when run=512); slot (c, p) owns windows starting at
    L - 1024*run + (c*128 + p)*run."""
    cols, width = geom(run)
    start0 = L - 1024 * run
    ff64 = np.float64(ff32)
    p = np.arange(128)
    t = np.arange(run)
    in_maps = []
    for c in range(NCORES):
        base = start0 + c * 128 * run
        xt = np.zeros((128, width), dtype=np.float32)
        xt[:, 0:cols] = np.lib.stride_tricks.as_strided(
            x[base:], shape=(128, cols), strides=(run * 4, 4)
        )
        # weight of window t of partition p: global index i = i0 + run-1-t
        # (np.power, not exp(log*expo): ff == 0.0 needs 0^0 == 1)
        i0 = L - 1 - (base + run * p + (run - 1))
        expo = (i0[:, None] + (run - 1 - t)[None, :]).astype(np.float64)
        xt[:, cols : cols + run] = (np.power(ff64, expo) / 127.0).astype(
            np.float32
        )
        in_maps.append({"xt": xt})
    return in_maps


def combine_host(accs: list[np.ndarray], ff32: np.float32) -> np.ndarray:
    """accs: per-core [128,64] tiles, partial sums in column 0. f64 host sum."""
    ff64 = np.float64(ff32)
    total = np.float64(0.0)
    for c in range(NCORES):
        total += np.asarray(accs[c])[:, 0].astype(np.float64).sum()
    norm = (1.0 - ff64) / (1.0 - np.power(ff64, np.float64(L)))
    return np.asarray(np.float32(norm * total))


def kernel(past_returns, features, raw_forgetting_factor):
    x = np.ascontiguousarray(np.asarray(past_returns, dtype=np.float32))
    assert x.shape == (N,), x.shape
    raw = np.float64(np.asarray(raw_forgetting_factor).reshape(-1)[0])
    ff32 = np.float32(1.0 / (1.0 + np.exp(-raw)))

    run = plan_run(np.float64(ff32))
    nc = _get_nc(run)
    in_maps = make_in_maps(x, ff32, run)
    res = run_bass_kernel_spmd(nc, in_maps, list(range(NCORES)))
    accs = [res.results[c]["acc"] for c in range(NCORES)]
    return combine_host(accs, ff32)


# revision 12
# speedup vs baseline: 1.0954x; 1.0604x over previous
"""EWMA predictor (sliding-window variance, exponentially weighted sum) on 8 trn2 cores.

Math: for j in [0, L): window_j = x[j : j+128], weight ff^(L-1-j),
result = norm * sum_j ff^(L-1-j) * var(window_j, ddof=1),
norm = (1-ff)/(1-ff^L), ff = sigmoid(raw_forgetting_factor).

Sharding: windows split over 8 cores x 128 partitions; partition p of core c
owns `run` consecutive windows and loads the run+127 input elements covering
them (halo overlap) plus a per-window weight row
WT[p, t] = ff^(i0(c,p) + run-1-t) / 127 in the trailing columns.

Device program (per core):
  input:  SWDGE dma_gather; row indices built on-device (Pool iotas + DVE
          int32 and/add/cast, replicated in every 16-partition group since
          each Q7 core reads the wrapped idxs from its own group), the
          descriptors prepared and triggered early so the completion
          latency overlaps the whole compute chain.
  chain:  all on DVE: X2 = x^2; one fused [128,2,W] reduce seeds both
          sliding-sum recurrences; two tensor_tensor_scan ops produce the
          128-window sums s1/s2; d = s2 - s1^2/128 (= 127*var_unbiased);
          contrib[p] = sum_t WT[p,t]*d[p,t] (elementwise mult + reduce).
  output: SWDGE dma_scatter_add writes contrib[p] into acc[p,0] (stride
          256B rows; output DRAM buffers are zero-initialized by both the
          native runner and the bass2jax/PJRT path).
Host sums the 8x128 partials and applies norm in float64.
"""

from contextlib import ExitStack

import numpy as np

import concourse.bass as bass
import concourse.mybir as mybir
from concourse import library_config
from concourse.library_overlay import lower_extended_insts
from concourse.bass_utils import run_bass_kernel_spmd

L = 524288          # look-back windows
W = 128             # variance window length
N = L + W           # input length
NCORES = 8
WIN_PER_CORE = L // NCORES      # 65536
RUN = WIN_PER_CORE // 128       # 512 windows per partition (full computation)

_NC_CACHE = {}


def plan_run(ff64: float) -> int:
    """Windows-per-partition for the adaptive program.

    Weights ff^i are EXACTLY zero in fp32 (past subnormals) once
    i > 104/|ln ff|, so windows beyond that cannot affect any output bit.
    Keep a >=1024-window margin, round the 1024*run window count up to a
    power-of-two run, clamp to [8, 512]; run=512 is the exact full
    computation (all L windows).
    """
    lnff = np.log(np.float64(ff64))
    if not (lnff < -1e-9):
        return RUN
    k_needed = 104.0 / (-lnff)
    run_min = int(np.ceil((k_needed + 1024.0) / 1024.0))
    run = 8
    while run < run_min:
        run *= 2
    return min(run, RUN)


def geom(run: int) -> tuple[int, int]:
    # row: [zero | x(cols) | WT(run) | T-block(128)]
    cols = run + W - 1
    width = (1 + cols + run + 128 + 63) // 64 * 64
    return cols, width


def build_nc(run: int = 8) -> bass.Bass:
    cols, width = geom(run)
    nc = bass.Bass(trn_type="TRN2")
    f32 = mybir.dt.float32
    i32 = mybir.dt.int32
    i16 = mybir.dt.int16
    A = mybir.AluOpType
    xt = nc.declare_dram_parameter("xt", [128, width], f32, isOutput=False)
    acc = nc.declare_dram_parameter("acc", [128, 64], f32, isOutput=True)

    with ExitStack() as stack:
        e = stack.enter_context
        XX = e(nc.sbuf_tensor([128, width], f32))
        T2T = e(nc.sbuf_tensor([128, 128], f32))
        S = e(nc.sbuf_tensor([128, 2 * run], f32))
        X2B = e(nc.sbuf_tensor([128, 2 * run], f32))
        ONES = e(nc.sbuf_tensor([128, 1], f32))
        PS = e(nc.psum_tensor([128, 2], f32))
        D = e(nc.sbuf_tensor([128, run], f32))
        E = e(nc.sbuf_tensor([128, run], f32))
        CONTRIB = e(nc.sbuf_tensor([128, 1], f32))
        C32 = e(nc.sbuf_tensor([128, 8], i32))
        P32 = e(nc.sbuf_tensor([128, 1], i32))
        V32 = e(nc.sbuf_tensor([128, 8], i32))
        IDX = e(nc.sbuf_tensor([128, 8], i16))
        DLY = e(nc.sbuf_tensor([128, 224], f32))
        PDLY = e(nc.sbuf_tensor([1, 1280], f32))
        gsem = e(nc.semaphore())    # gather-in completion (DMA sem)
        scsem = e(nc.semaphore())   # scatter-out completion (DMA sem)
        psem = e(nc.semaphore())    # pool setup progress
        esem = e(nc.semaphore())    # gather trigger enqueued
        vsem = e(nc.semaphore())    # DVE progress (idx build + chain)
        msem = e(nc.semaphore())    # PE matmul progress
        block = e(nc.Block())
        X = XX[:, 0:width]
        WT = XX[:, 1 + cols : 1 + cols + run]
        TB = XX[:, 1 + cols + run : 1 + cols + run + 128]
        X2LO = X2B[:, 0:run]
        X2HI = X2B[:, run : 2 * run]
        S1 = S[:, 0:run]
        S2 = S[:, run : 2 * run]

        @block.gpsimd
        def _(gpsimd):
            # Gather row index ingredients: IDX[p, c] must be (p%16) + 16c in
            # every 16-partition group (each Q7 core reads the wrapped idxs
            # from its own group; the interpreter reads group 0, hardware
            # group 1). Pool emits the affine parts, DVE the bitwise ones
            # (walrus: int bitwise ops are DVE-only, int16 alu is DVE-only).
            gpsimd.iota(
                C32[:], pattern=[[16, 8]], base=0, channel_multiplier=0
            ).then_inc(psem, 1)
            gpsimd.iota(
                P32[:], pattern=[[0, 1]], base=0, channel_multiplier=1
            ).then_inc(psem, 1)
            gpsimd.memset(PDLY[:, 0:64], 0)
            gpsimd.memset(PDLY[:, 64:116], 0)
            gpsimd.wait_ge(psem, 2)
            gpsimd.wait_ge(vsem, 1)        # P32 &= 15 done on DVE
            gpsimd.tensor_tensor(
                V32[:], C32[:], P32[:].broadcast_to([128, 8]), op=A.add
            ).then_inc(psem, 1)
            gpsimd.wait_ge(psem, 3)
            gpsimd.tensor_copy(IDX[:], V32[:]).then_inc(psem, 1)
            gpsimd.load_library(library_config.mlp)
            gpsimd.wait_ge(psem, 4)
            gpsimd.dma_gather(
                X.unsqueeze(1), xt[:], IDX[:], num_idxs=128, num_idxs_reg=128,
                elem_size=width, prepare_only=True, sem=gsem,
            ).then_inc(psem, 1)
            gpsimd.wait_ge(psem, 5)
            gpsimd.trigger_dma(1)          # fire gather-in
            gpsimd.sem_inc(esem, 1)        # consumers may now wait on gsem
            gpsimd.dma_scatter_add(
                acc[0:128, 0:1], CONTRIB[:], IDX[:], num_idxs=128,
                num_idxs_reg=128, elem_size=1, elem_step=64,
                prepare_only=True, sem=scsem,
            ).then_inc(psem, 1)
            gpsimd.wait_ge(psem, 6)
            # self-delay: arrive at the vsem wait just after the run=8 chain's
            # last enqueue (eager pass); longer chains fall back to the normal
            # blocked wake (+31ns), shorter configs never occur.
            gpsimd.memset(PDLY[:, 128:1028], 0)
            gpsimd.wait_ge(vsem, 15)       # contrib ready
            gpsimd.trigger_dma(1)          # fire scatter-out
            gpsimd.wait_ge(scsem, 16)      # real-HW: scatter landed before exit

        @block.vector
        def _(vector):
            # idx build step 1 (bitwise is DVE-only): P32 &= 15
            vector.wait_ge(psem, 2)
            vector.tensor_scalar(
                P32[:], P32[:], 15, None, op0=A.bitwise_and
            ).then_inc(vsem, 1)
            vector.memset(ONES[:], 1.0).then_inc(vsem, 1)
            vector.memset(X2LO[:, 0:1], 0.0).then_inc(vsem, 1)
            # self-delay sized to arrive at the waits just after the Pool
            # trigger enqueues the gather (eager sem pass); if the trigger is
            # later than this, the blocked wait wakes normally (+100ns).
            vector.memset(DLY[:], 0.0)

            # compute chain. Initial window sums come from PE: PS[:,0]/PS[:,1]
            # hold sum of each window's first 127 elements (x and x^2), so the
            # scans produce s[0] in their first step and write [0:run] fully.
            vector.wait_ge(esem, 1)
            vector.wait_ge(gsem, 16)
            vector.scalar_tensor_tensor(
                T2T[:], TB[:], 1.0, TB[:], op0=A.mult, op1=A.mult
            ).then_inc(vsem, 4)
            vector.wait_ge(vsem, 7)
            vector.scalar_tensor_tensor(
                X2LO[:, 1:run], X[:, 1:run], 1.0, X[:, 1:run],
                op0=A.mult, op1=A.mult,
            ).then_inc(vsem, 1)
            vector.wait_ge(vsem, 8)
            vector.wait_ge(msem, 1)
            # s1[t] = (x[W-1+t] + state) - [0, x[0..run-2]][t]
            vector.tensor_tensor_scan(
                S1[:], X[:, W : W + run], X[:, 0:run],
                initial=PS[:, 0:1], op0=A.add, op1=A.subtract,
            ).then_inc(vsem, 1)
            vector.wait_ge(vsem, 9)
            vector.scalar_tensor_tensor(
                X2HI[:], X[:, W : W + run], 1.0, X[:, W : W + run],
                op0=A.mult, op1=A.mult,
            ).then_inc(vsem, 1)
            vector.wait_ge(vsem, 10)
            vector.wait_ge(msem, 2)
            vector.tensor_tensor_scan(
                S2[:], X2HI[:], X2LO[:],
                initial=PS[:, 1:2], op0=A.add, op1=A.subtract,
            ).then_inc(vsem, 1)
            vector.wait_ge(vsem, 11)
            # d = s2 - s1^2/128  (D = (s1 * -1/128) * s1; D = D + s2)
            vector.scalar_tensor_tensor(
                D[:], S1[:], -1.0 / 128.0, S1[:], op0=A.mult, op1=A.mult
            ).then_inc(vsem, 1)
            vector.wait_ge(vsem, 12)
            vector.scalar_tensor_tensor(
                D[:], D[:], 1.0, S2[:], op0=A.mult, op1=A.add
            ).then_inc(vsem, 1)
            vector.wait_ge(vsem, 13)
            # contrib[p] = sum_t WT[p,t] * d[p,t]
            vector.scalar_tensor_tensor(
                E[:], D[:], 1.0, WT[:], op0=A.mult, op1=A.mult
            ).then_inc(vsem, 1)
            vector.wait_ge(vsem, 14)
            vector.reduce_sum(
                CONTRIB[:], E[:], axis=mybir.AxisListType.X
            ).then_inc(vsem, 1)

        @block.tensor
        def _(pe):
            pe.wait_ge(esem, 1)
            pe.wait_ge(gsem, 16)
            pe.wait_ge(vsem, 2)            # ONES ready
            pe.matmul(
                PS[:, 0:1], TB[0:127, :], ONES[0:127, 0:1]
            ).then_inc(msem, 1)
            pe.wait_ge(vsem, 7)            # T2T ready
            pe.matmul(
                PS[:, 1:2], T2T[0:127, :], ONES[0:127, 0:1]
            ).then_inc(msem, 1)

    lower_extended_insts(nc)  # encode ISA bytes for the NEFF compiler
    return nc


def _get_nc(run: int) -> bass.Bass:
    if run not in _NC_CACHE:
        _NC_CACHE[run] = build_nc(run)
    return _NC_CACHE[run]


def make_in_maps(
    x: np.ndarray, ff32: np.float32, run: int
) -> list[dict[str, np.ndarray]]:
    """Per-core input tiles covering the last 1024*run windows (all L windows
    when run=512); slot (c, p) owns windows starting at
    L - 1024*run + (c*128 + p)*run."""
    cols, width = geom(run)
    start0 = L - 1024 * run
    ff64 = np.float64(ff32)
    p = np.arange(128)
    t = np.arange(run)
    in_maps = []
    for c in range(NCORES):
        base = start0 + c * 128 * run
        xt = np.zeros((128, width), dtype=np.float32)
        xt[:, 1 : 1 + cols] = np.lib.stride_tricks.as_strided(
            x[base:], shape=(128, cols), strides=(run * 4, 4)
        )
        # weight of window t of partition p: global index i = i0 + run-1-t
        # (np.power, not exp(log*expo): ff == 0.0 needs 0^0 == 1)
        i0 = L - 1 - (base + run * p + (run - 1))
        expo = (i0[:, None] + (run - 1 - t)[None, :]).astype(np.float64)
        xt[:, 1 + cols : 1 + cols + run] = (np.power(ff64, expo) / 127.0).astype(
            np.float32
        )
        # T block: T[q, j] = x[base + j*run + q] (window j's q-th element)
        xt[:, 1 + cols + run : 1 + cols + run + 128] = (
            np.lib.stride_tricks.as_strided(
                x[base:], shape=(128, 128), strides=(4, run * 4)
            )
        )
        in_maps.append({"xt": xt})
    return in_maps


def combine_host(accs: list[np.ndarray], ff32: np.float32) -> np.ndarray:
    """accs: per-core [128,64] tiles, partial sums in column 0. f64 host sum."""
    ff64 = np.float64(ff32)
    total = np.float64(0.0)
    for c in range(NCORES):
        total += np.asarray(accs[c])[:, 0].astype(np.float64).sum()
    norm = (1.0 - ff64) / (1.0 - np.power(ff64, np.float64(L)))
    return np.asarray(np.float32(norm * total))


def kernel(past_returns, features, raw_forgetting_factor):
    x = np.ascontiguousarray(np.asarray(past_returns, dtype=np.float32))
    assert x.shape == (N,), x.shape
    raw = np.float64(np.asarray(raw_forgetting_factor).reshape(-1)[0])
    ff32 = np.float32(1.0 / (1.0 + np.exp(-raw)))

    run = plan_run(np.float64(ff32))
    nc = _get_nc(run)
    in_maps = make_in_maps(x, ff32, run)
    res = run_bass_kernel_spmd(nc, in_maps, list(range(NCORES)))
    accs = [res.results[c]["acc"] for c in range(NCORES)]
    return combine_host(accs, ff32)
